# revision 30
# baseline (speedup 1.0000x reference)
"""Trainium2 Bass kernel for nn_AttentionSE3 (graph attention message passing).

Strategy (edge/graph parallel, fully host-prepped ELL layout):
- Attention is a segment softmax over incoming edges of each dst node.  Logits are
  dot(k_edge, q_dst)/sqrt(128) with k,q ~ N(0,1): |logit| <~ 2, so the max-subtraction
  is dropped (softmax is shift-invariant; exp() never overflows here) and
  out[n] = sum_e exp(logit_e) * v_e / sum_e exp(logit_e).
- Host sorts nodes by in-degree and packs them into 128-node blocks; blocks are
  dealt round-robin to the 8 cores ("positions").  A device group merges GB
  consecutive positions sharing one even capacity D (cost model below trades the
  padding of merging against ~1.5us of per-group instruction overhead).  Every
  core runs the same static program (no collectives: no node's edges ever span
  two cores).
- Per (node, d) "slot" the host gathers the edge's key row [128] and value row
  [96] (zero for padding).  A padded slot contributes exactly exp(0)=1 to the
  softmax denominator, so the device subtracts a per-node pad count (exact
  correction).  Zero-degree nodes get pad_count = D-1 so the denominator is
  exactly 1 and the output row is 0, matching segment_sum semantics.
- ALL compute stays on VectorE + ScalarE.  GPSIMD shares an SBUF port with
  VectorE; measured on HW, a DVE tensor_tensor slows down 7-9x while any GPSIMD
  tensor op runs, so offloading elementwise work to GPSIMD is a large net loss.
- k loads issue on the SP HWDGE queue and v loads on the ACT HWDGE queue: one
  logical DMA queue tops out at ~165 GB/s, two run in parallel.
- The merged sub-blocks are interleaved at the INNERMOST axis: key tiles are
  (kk16, d, h8, s), value tiles (d, cx12, h8, s), queries (kk, h, s), so every
  per-group op is a single instruction with <=3 free AP dims (4+-dim APs
  measured ~20% slower) regardless of GB:
    * k *= q broadcasts q over the middle d dim (bf16 2x, in place),
    * the dk-reduction is a 4-level halving tree of fully contiguous adds
      (tensor_reduce is 1x / ~1.8 cyc/elem strided; the tree runs ~0.6),
    * v *= expw broadcasts expw over the middle cx dim (bf16 2x, in place),
    * the d-reduction halves in place over the leading d axis (fully
      contiguous) while the depth is even, then one strided reduce over the
      odd remainder.
- Normalization is deferred and chunked: a few groups at a time get the
  denominator subtract/reciprocal/scale and their output store, so only the
  last chunk's work sits on the critical-path tail.
"""

import numpy as np

import concourse.bacc as bacc
import concourse.mybir as mybir
from concourse import tile
from concourse.bass_utils import run_bass_kernel_spmd

try:
    import ml_dtypes
    BF16_NP = np.dtype(ml_dtypes.bfloat16)
except ImportError:  # pragma: no cover
    BF16_NP = None

N_NODES = 50000
H = 8
P = 128   # nodes per block
N_CORES = 8
SCALE = float(1.0 / np.sqrt(128.0))
F32 = mybir.dt.float32

W_CAP = 96      # max slots/partition per group (SBUF tile budget)
MERGE_NS = 1500.0   # overhead saved per merged position
SLOT_NS = 281.0     # cost of one pad slot-unit (per partition)
N_CHUNKS = 5        # normalization / store chunks

# key columns permuted from [h(8), kk(16)] to [kk(16), h(8)]
PERM_K = np.arange(128).reshape(8, 16).T.reshape(-1)
# value columns permuted from [h(8), cx(12)] to [cx(12), h(8)]
PERM_V = np.arange(96).reshape(8, 12).T.reshape(-1)


def _even(x):
    return (int(x) + 1) // 2 * 2


def _chain_ns(D, GB):
    """d-reduction cost (ns/partition): flat halves at 0.53 cyc/elem while the
    depth is even, then (r-1) flat accumulate-adds over the odd remainder."""
    c, x = 0.0, D
    while x > 1 and x % 2 == 0:
        x //= 2
        c += 0.53 * 96 * GB * x
    if x > 1:
        c += (x - 1) * (0.53 * 96 * GB + 160 * 0.96)
    return c / 0.96


def _plan_groups(Dpos):
    """Greedy merge of consecutive block-positions into groups, then round each
    capacity up where that makes the d-reduction chain cheaper.

    Returns list of (start_pos, GB, D) with D even, GB*D <= W_CAP.
    """
    ng = len(Dpos)
    plan = []
    i = 0
    while i < ng:
        gb = 1
        while i + gb < ng:
            cand = Dpos[i:i + gb + 1]
            D = _even(max(cand))
            if (gb + 1) * D > W_CAP:
                break
            pad = SLOT_NS * ((gb + 1) * D - sum(_even(x) for x in cand))
            if pad >= MERGE_NS * gb:
                break
            gb += 1
        D = _even(max(Dpos[i:i + gb]))
        best, best_c = D, _chain_ns(D, gb)
        for Dp in range(D + 2, D + 10, 2):
            if Dp * gb > max(W_CAP, D * gb):
                break
            c = SLOT_NS * gb * (Dp - D) + _chain_ns(Dp, gb)
            if c < best_c:
                best, best_c = Dp, c
        plan.append((i, gb, best))
        i += gb
    # Execution order: the two smallest tiles first (cheap loads -> compute
    # starts early), then descending so the tail group is small too.
    plan.sort(key=lambda t: t[1] * t[2])
    plan = plan[:2] + plan[2:][::-1]
    return plan


# ---------------------------------------------------------------- host prep

def prepare(value, key, query0, query1, edge_index, n_nodes=N_NODES, n_cores=N_CORES):
    """Build per-core padded ELL shards.  Returns (in_maps, meta)."""
    value = np.asarray(value, dtype=np.float32)
    key = np.asarray(key, dtype=np.float32)
    query0 = np.asarray(query0, dtype=np.float32)
    query1 = np.asarray(query1, dtype=np.float32)
    n_edges = key.shape[0]

    dst = np.asarray(edge_index[1], dtype=np.int64)
    deg = np.bincount(dst, minlength=n_nodes).astype(np.int64)
    unit = P * n_cores
    n_pad = -(-n_nodes // unit) * unit
    deg_pad = np.concatenate([deg, np.zeros(n_pad - n_nodes, dtype=np.int64)])
    nb = n_pad // P          # total blocks
    ng = nb // n_cores       # block positions per core

    order = np.argsort(deg_pad, kind="stable")  # node ids, degree-ascending
    degs_o = deg_pad[order]

    blk_max = degs_o.reshape(nb, P).max(axis=1)
    Dpos = np.maximum(blk_max.reshape(ng, n_cores).max(axis=1), 1).astype(np.int64)
    plan = _plan_groups(list(Dpos))

    # per-position group index / sub index / capacity
    g_of_pos = np.zeros(ng, dtype=np.int64)
    sub_of_pos = np.zeros(ng, dtype=np.int64)
    D_of_pos = np.zeros(ng, dtype=np.int64)
    for gi, (p0, gb, D) in enumerate(plan):
        g_of_pos[p0:p0 + gb] = gi
        sub_of_pos[p0:p0 + gb] = np.arange(gb)
        D_of_pos[p0:p0 + gb] = D
    GBs = np.array([gb for _, gb, _ in plan], dtype=np.int64)
    Ds = np.array([D for _, _, D in plan], dtype=np.int64)
    off = np.concatenate([[0], np.cumsum(P * GBs * Ds)]).astype(np.int64)
    S = int(off[-1])  # slot rows per core

    pos = np.arange(n_pad)
    block = pos // P
    core_of = block % n_cores
    p_of = block // n_cores
    row = pos % P
    gg = g_of_pos[p_of]
    Dg = Ds[gg]
    # s-major scatter layout within a group; re-packed to interleaved below
    base = off[gg] + row * (GBs[gg] * Dg) + sub_of_pos[p_of] * Dg

    edge_order = np.argsort(dst, kind="stable")
    starts = np.concatenate([[0], np.cumsum(deg)])

    pp = np.repeat(pos, degs_o)
    cum0 = np.concatenate([[0], np.cumsum(degs_o)])[:-1]
    d_idx = np.arange(n_edges) - np.repeat(cum0, degs_o)
    node_of_pp = order[pp]
    edge_ids = edge_order[starts[node_of_pp] + d_idx]
    slot_global = core_of[pp] * S + base[pp] + d_idx

    kp = np.zeros((n_cores * S, 128), dtype=np.float32)
    kp[slot_global] = key[:, PERM_K][edge_ids]
    vp = np.zeros((n_cores * S, 96), dtype=np.float32)
    vp[slot_global] = value.reshape(n_edges, 96)[:, PERM_V][edge_ids]
    kp = kp.reshape(n_cores, S, 128)
    vp = vp.reshape(n_cores, S, 96)

    # Re-pack each group's per-node block from (s, d, feat) to the
    # innermost-interleaved device layouts:
    #   keys   (kk, d, h, s)   values (d, cx, h, s)
    for gi, (p0, gb, D) in enumerate(plan):
        s0, s1 = int(off[gi]), int(off[gi + 1])
        kb = kp[:, s0:s1].reshape(n_cores, P, gb, D, 16, 8)
        kp[:, s0:s1] = np.ascontiguousarray(
            kb.transpose(0, 1, 4, 3, 5, 2)).reshape(n_cores, s1 - s0, 128)
        vb = vp[:, s0:s1].reshape(n_cores, P, gb, D, 12, 8)
        vp[:, s0:s1] = np.ascontiguousarray(
            vb.transpose(0, 1, 3, 4, 5, 2)).reshape(n_cores, s1 - s0, 96)

    qfull = np.concatenate([query0, query1], axis=-1).reshape(n_nodes, 128)[:, PERM_K]
    q_pad = np.zeros((n_pad, 128), dtype=np.float32)
    q_pad[:n_nodes] = qfull
    q_sorted = q_pad[order].reshape(nb, P, 128)

    pc = (Dg - degs_o).astype(np.float32)
    zero_deg = degs_o == 0
    pc[zero_deg] = (Dg[zero_deg] - 1).astype(np.float32)
    pc_sorted = pc.reshape(nb, P)

    dt = BF16_NP
    kp = kp.astype(dt)
    vp = vp.astype(dt)
    in_maps = []
    for c in range(n_cores):
        qc = q_sorted[c::n_cores]              # [ng, P, 128], (kk, h) cols
        pcc = pc_sorted[c::n_cores]            # [ng, P]
        # queries per group in (kk, h, s) order; pad counts in (h, s) order
        q_c = np.zeros((P, ng * 128), dtype=np.float32)
        pc_c = np.zeros((P, ng * H), dtype=np.float32)
        for gi, (p0, gb, D) in enumerate(plan):
            qg = qc[p0:p0 + gb].reshape(gb, P, 16, 8)        # (s, P, kk, h)
            q_c[:, p0 * 128:(p0 + gb) * 128] = (
                qg.transpose(1, 2, 3, 0).reshape(P, gb * 128))
            pg = pcc[p0:p0 + gb]                             # (s, P)
            pc_c[:, p0 * H:(p0 + gb) * H] = np.repeat(
                pg.T[:, None, :], H, axis=1).reshape(P, H * gb)
        in_maps.append({"kp": kp[c], "vp": vp[c],
                        "q": q_c.astype(dt), "pc": pc_c})

    meta = dict(plan=plan, off=off, S=S, NG=len(plan), ng=ng, NB=nb,
                order=order, n_nodes=n_nodes, n_pad=n_pad)
    return in_maps, meta


def unshard_output(out_cores, meta):
    """out_cores: list of [128, ng*96] in (c, h, s) group order -> [N, 32, 3]."""
    nb, ng = meta["NB"], meta["ng"]
    plan = meta["plan"]
    n_cores = len(out_cores)
    order, n_nodes, n_pad = meta["order"], meta["n_nodes"], meta["n_pad"]
    out_sorted = np.zeros((nb, P, 96), dtype=np.float32)
    perm_v_inv = np.argsort(PERM_V)
    for c in range(n_cores):
        oc = out_cores[c]                      # [P, ng*96]
        po = np.zeros((ng, P, 96), dtype=np.float32)
        for gi, (p0, gb, D) in enumerate(plan):
            blk = oc[:, p0 * 96:(p0 + gb) * 96].reshape(P, 12, 8, gb)
            po[p0:p0 + gb] = blk.transpose(3, 0, 1, 2).reshape(gb, P, 96)
        out_sorted[c::n_cores] = po
    out_sorted = out_sorted.reshape(n_pad, 96)[:, perm_v_inv]
    out_full = np.zeros((n_nodes, 96), dtype=np.float32)
    mask = order < n_nodes
    out_full[order[mask]] = out_sorted[mask]
    return out_full.reshape(n_nodes, 32, 3)


# ---------------------------------------------------------------- bass kernel

def build(plan, S, ng, n_cores=N_CORES):
    NG = len(plan)
    off = np.concatenate(
        [[0], np.cumsum([P * gb * D for _, gb, D in plan])]).astype(np.int64)

    nc = bacc.Bacc("TRN2", target_bir_lowering=False, debug=False,
                   num_devices=n_cores)
    DT = mybir.dt.bfloat16
    kp = nc.declare_dram_parameter("kp", [S, 128], DT, isOutput=False)
    vp = nc.declare_dram_parameter("vp", [S, 96], DT, isOutput=False)
    q = nc.declare_dram_parameter("q", [P, ng * 128], DT, isOutput=False)
    pc = nc.declare_dram_parameter("pc", [P, ng * H], F32, isOutput=False)
    out = nc.declare_dram_parameter("out", [P, ng * 96], F32, isOutput=True)

    mult = mybir.AluOpType.mult
    add = mybir.AluOpType.add
    AX = mybir.AxisListType.X

    with tile.TileContext(nc) as tc:
        with tc.tile_pool(name="res", bufs=1) as res, \
             tc.tile_pool(name="ld", bufs=2) as ld, \
             tc.tile_pool(name="small", bufs=2) as small:
            q_sb = res.tile([P, ng * 128], DT)
            pc_sb = res.tile([P, ng * H], F32)
            out_sb = res.tile([P, ng * 96], F32)
            ss_all = res.tile([P, ng * H], F32)
            rs_all = res.tile([P, ng * H], F32)

            for g in range(NG):
                p0, GB, D = plan[g]
                W = GB * D           # slots per partition
                HS = H * GB          # interleaved (h, s) width
                s0 = int(off[g])
                # alternate k/v loads between the SP and ACT HWDGE queues
                eng_a = nc.sync if g % 2 == 0 else nc.scalar
                eng_b = nc.scalar if g % 2 == 0 else nc.sync
                # per-group query / pad-count slices first (tiny, FIFO per
                # queue: they must not sit behind the big k/v transfers)
                nc.scalar.dma_start(q_sb[:, p0 * 128:(p0 + GB) * 128],
                                    q[:, p0 * 128:(p0 + GB) * 128])
                nc.sync.dma_start(pc_sb[:, p0 * H:(p0 + GB) * H],
                                  pc[:, p0 * H:(p0 + GB) * H])
                kt = ld.tile([P, W * 128], DT, tag="kt", bufs=2)
                eng_a.dma_start(
                    kt[:], kp[s0:s0 + P * W, :].rearrange("(n w) f -> n (w f)", n=P))
                vt = ld.tile([P, W * 96], DT, tag="vt")
                eng_b.dma_start(
                    vt[:], vp[s0:s0 + P * W, :].rearrange("(n w) f -> n (w f)", n=P))

                # k *= q in place (content (kk, d, h, s); q is (kk, h, s) and
                # broadcasts over the middle d dim; bf16 2x, 3 free AP dims)
                qb = (q_sb[:, p0 * 128:(p0 + GB) * 128]
                      .rearrange("n (kk hs) -> n kk hs", kk=16)
                      .unsqueeze(2).broadcast_to([P, 16, D, HS]))
                k3 = kt[:].rearrange("n (kk d hs) -> n kk d hs", kk=16, d=D)
                nc.vector.tensor_tensor(out=k3, in0=k3, in1=qb, op=mult)

                # dk-halving tree: every level one fully contiguous add
                src, sw = kt, 128
                for X, tag in ((64, "t64"), (32, "t32"), (16, "t16")):
                    dstt = small.tile([P, W * X], DT, tag=tag, bufs=3)
                    nc.vector.tensor_tensor(
                        out=dstt[:], in0=src[:, :W * X], in1=src[:, W * X:W * sw],
                        op=add)
                    src, sw = dstt, X
                lg = small.tile([P, W * H], DT, tag="lg", bufs=3)
                nc.vector.tensor_tensor(
                    out=lg[:], in0=src[:, :W * H], in1=src[:, W * H:], op=add)

                # expw = exp(scale * logits)  (ScalarE; contiguous)
                ew = small.tile([P, W * H], DT, tag="ew", bufs=3)
                nc.scalar.activation(out=ew[:], in_=lg[:],
                                     func=mybir.ActivationFunctionType.Exp,
                                     scale=SCALE)

                # v *= expw in place (content (d, cx, h, s); expw is (d, h, s)
                # and broadcasts over the middle cx dim; bf16 2x, 3 free dims)
                eb = (ew[:].rearrange("n (d hs) -> n d hs", d=D)
                      .unsqueeze(2).broadcast_to([P, D, 12, HS]))
                v3 = vt[:].rearrange("n (d c hs) -> n d c hs", d=D, c=12)
                nc.vector.tensor_tensor(out=v3, in0=v3, in1=eb, op=mult)

                # segment sum of expw over d -> (h, s) per group: one flat
                # in-place halve (wv has consumed ew) then a strided reduce
                rs = D
                if rs % 2 == 0:
                    rs //= 2
                    nc.vector.tensor_tensor(
                        out=ew[:, :rs * HS], in0=ew[:, :rs * HS],
                        in1=ew[:, rs * HS:], op=add)
                nc.vector.tensor_reduce(
                    out=ss_all[:, p0 * H:(p0 + GB) * H],
                    in_=ew[:, :rs * HS].rearrange("n (d hs) -> n hs d", d=rs),
                    axis=AX, op=add)

                # d-reduction: halve in place over the leading d axis (fully
                # contiguous) while even, then strided reduce the remainder
                og = out_sb[:, p0 * 96:(p0 + GB) * 96]
                r = D
                FB = 96 * GB  # elems per d-layer
                while r > 1 and r % 2 == 0:
                    r //= 2
                    if r == 1:
                        nc.vector.tensor_tensor(
                            out=og, in0=vt[:, :FB], in1=vt[:, FB:2 * FB], op=add)
                    else:
                        nc.vector.tensor_tensor(
                            out=vt[:, :r * FB], in0=vt[:, :r * FB],
                            in1=vt[:, r * FB:2 * r * FB], op=add)
                # odd remainder: accumulate with flat 2x adds (a strided
                # tensor_reduce runs at ~1.8 cyc/elem, 3.5x slower)
                for j in range(1, r - 1):
                    nc.vector.tensor_tensor(
                        out=vt[:, :FB], in0=vt[:, :FB],
                        in1=vt[:, j * FB:(j + 1) * FB], op=add)
                if r > 1:
                    nc.vector.tensor_tensor(
                        out=og, in0=vt[:, :FB],
                        in1=vt[:, (r - 1) * FB:r * FB], op=add)

                # normalization + store per group: stores spread over the run
                # so only the last group's tail is exposed
                ca, cb = p0 * H, (p0 + GB) * H
                nc.vector.tensor_sub(out=ss_all[:, ca:cb], in0=ss_all[:, ca:cb],
                                     in1=pc_sb[:, ca:cb])
                nc.vector.reciprocal(out=rs_all[:, ca:cb], in_=ss_all[:, ca:cb])
                nc.vector.tensor_tensor(
                    out=og.rearrange("n (c hs) -> n c hs", c=12),
                    in0=og.rearrange("n (c hs) -> n c hs", c=12),
                    in1=(rs_all[:, ca:cb]
                         .unsqueeze(1).broadcast_to([P, 12, GB * H])),
                    op=mult)
                eng_a.dma_start(out[:, p0 * 96:(p0 + GB) * 96], og)

    nc.compile()
    return nc


# ---------------------------------------------------------------- entry point

LAST_RESULT = None  # BassKernelResults of the most recent run (for test harness)


def kernel(value, key, query0, query1, edge_index):
    global LAST_RESULT
    import os
    in_maps, meta = prepare(value, key, query0, query1, edge_index)
    nc = build(meta["plan"], meta["S"], meta["ng"])
    res = run_bass_kernel_spmd(nc, in_maps, list(range(N_CORES)),
                               tmpdir=os.environ.get("BASS_SPMD_TMPDIR"))
    LAST_RESULT = res
    out_cores = [res.results[c]["out"] for c in range(N_CORES)]
    return unshard_output(out_cores, meta)


# revision 31
# speedup vs baseline: 1.8781x; 1.8781x over previous
"""Trainium2 Bass kernel for nn_AttentionSE3 (graph attention message passing).

Strategy (edge/graph parallel, fully host-prepped ELL layout):
- Attention is a segment softmax over incoming edges of each dst node.  Logits are
  dot(k_edge, q_dst)/sqrt(128) with k,q ~ N(0,1): |logit| <~ 2, so the max-subtraction
  is dropped (softmax is shift-invariant; exp() never overflows here) and
  out[n] = sum_e exp(logit_e) * v_e / sum_e exp(logit_e).
- Host sorts nodes by in-degree and packs them into 128-node blocks; blocks are
  dealt round-robin to the 8 cores ("positions").  A device group merges GB
  consecutive positions sharing one even capacity D (cost model below trades the
  padding of merging against ~1.5us of per-group instruction overhead).  Every
  core runs the same static program (no collectives: no node's edges ever span
  two cores).
- Per (node, d) "slot" the host gathers the edge's key row [128] and value row
  [96] (zero for padding).  A padded slot contributes exactly exp(0)=1 to the
  softmax denominator, so the device subtracts a per-node pad count (exact
  correction).  Zero-degree nodes get pad_count = D-1 so the denominator is
  exactly 1 and the output row is 0, matching segment_sum semantics.
- ALL compute stays on VectorE + ScalarE.  GPSIMD shares an SBUF port with
  VectorE; measured on HW, a DVE tensor_tensor slows down 7-9x while any GPSIMD
  tensor op runs, so offloading elementwise work to GPSIMD is a large net loss.
- k loads issue on the SP HWDGE queue and v loads on the ACT HWDGE queue: one
  logical DMA queue tops out at ~165 GB/s, two run in parallel.
- The merged sub-blocks are interleaved at the INNERMOST axis: key tiles are
  (kk16, d, h8, s), value tiles (d, cx12, h8, s), queries (kk, h, s), so every
  per-group op is a single instruction with <=3 free AP dims (4+-dim APs
  measured ~20% slower) regardless of GB:
    * k *= q broadcasts q over the middle d dim (bf16 2x, in place),
    * the dk-reduction is a 4-level halving tree of fully contiguous adds
      (tensor_reduce is 1x / ~1.8 cyc/elem strided; the tree runs ~0.6),
    * v *= expw broadcasts expw over the middle cx dim (bf16 2x, in place),
    * the d-reduction halves in place over the leading d axis (fully
      contiguous) while the depth is even, then one strided reduce over the
      odd remainder.
- Normalization is deferred and chunked: a few groups at a time get the
  denominator subtract/reciprocal/scale and their output store, so only the
  last chunk's work sits on the critical-path tail.
"""

import numpy as np

import concourse.bacc as bacc
import concourse.mybir as mybir
from concourse import tile
from concourse.bass_utils import run_bass_kernel_spmd

try:
    import ml_dtypes
    BF16_NP = np.dtype(ml_dtypes.bfloat16)
except ImportError:  # pragma: no cover
    BF16_NP = None

N_NODES = 50000
H = 8
P = 128   # nodes per block
N_CORES = 8
SCALE = float(1.0 / np.sqrt(128.0))
F32 = mybir.dt.float32

W_CAP = 96      # max slots/partition per group (SBUF tile budget)
MERGE_NS = 1500.0   # overhead saved per merged position
SLOT_NS = 281.0     # cost of one pad slot-unit (per partition)
N_CHUNKS = 5        # normalization / store chunks

# key columns permuted from [h(8), kk(16)] to [kk(16), h(8)]
PERM_K = np.arange(128).reshape(8, 16).T.reshape(-1)
# value columns permuted from [h(8), cx(12)] to [cx(12), h(8)]
PERM_V = np.arange(96).reshape(8, 12).T.reshape(-1)


def _even(x):
    return (int(x) + 1) // 2 * 2


def _chain_ns(D, GB):
    """d-reduction cost (ns/partition): flat halves at 0.53 cyc/elem while the
    depth is even, then (r-1) flat accumulate-adds over the odd remainder."""
    c, x = 0.0, D
    while x > 1 and x % 2 == 0:
        x //= 2
        c += 0.53 * 96 * GB * x
    if x > 1:
        c += (x - 1) * (0.53 * 96 * GB + 160 * 0.96)
    return c / 0.96


def _plan_groups(Dpos):
    """Greedy merge of consecutive block-positions into groups, then round each
    capacity up where that makes the d-reduction chain cheaper.

    Returns list of (start_pos, GB, D) with D even, GB*D <= W_CAP.
    """
    ng = len(Dpos)
    plan = []
    i = 0
    while i < ng:
        gb = 1
        while i + gb < ng:
            cand = Dpos[i:i + gb + 1]
            D = _even(max(cand))
            if (gb + 1) * D > W_CAP:
                break
            pad = SLOT_NS * ((gb + 1) * D - sum(_even(x) for x in cand))
            if pad >= MERGE_NS * gb:
                break
            gb += 1
        D = _even(max(Dpos[i:i + gb]))
        best, best_c = D, _chain_ns(D, gb)
        for Dp in range(D + 2, D + 10, 2):
            if Dp * gb > max(W_CAP, D * gb):
                break
            c = SLOT_NS * gb * (Dp - D) + _chain_ns(Dp, gb)
            if c < best_c:
                best, best_c = Dp, c
        plan.append((i, gb, best))
        i += gb
    # Execution order: the two smallest tiles first (cheap loads -> compute
    # starts early), then descending so the tail group is small too.
    plan.sort(key=lambda t: t[1] * t[2])
    plan = plan[:2] + plan[2:][::-1]
    return plan


# ---------------------------------------------------------------- host prep

def prepare(value, key, query0, query1, edge_index, n_nodes=N_NODES, n_cores=N_CORES):
    """Build per-core padded ELL shards.  Returns (in_maps, meta)."""
    value = np.asarray(value, dtype=np.float32)
    key = np.asarray(key, dtype=np.float32)
    query0 = np.asarray(query0, dtype=np.float32)
    query1 = np.asarray(query1, dtype=np.float32)
    n_edges = key.shape[0]

    dst = np.asarray(edge_index[1], dtype=np.int64)
    deg = np.bincount(dst, minlength=n_nodes).astype(np.int64)
    unit = P * n_cores
    n_pad = -(-n_nodes // unit) * unit
    deg_pad = np.concatenate([deg, np.zeros(n_pad - n_nodes, dtype=np.int64)])
    nb = n_pad // P          # total blocks
    ng = nb // n_cores       # block positions per core

    order = np.argsort(deg_pad, kind="stable")  # node ids, degree-ascending
    degs_o = deg_pad[order]

    blk_max = degs_o.reshape(nb, P).max(axis=1)
    Dpos = np.maximum(blk_max.reshape(ng, n_cores).max(axis=1), 1).astype(np.int64)
    plan = _plan_groups(list(Dpos))

    # per-position group index / sub index / capacity
    g_of_pos = np.zeros(ng, dtype=np.int64)
    sub_of_pos = np.zeros(ng, dtype=np.int64)
    D_of_pos = np.zeros(ng, dtype=np.int64)
    for gi, (p0, gb, D) in enumerate(plan):
        g_of_pos[p0:p0 + gb] = gi
        sub_of_pos[p0:p0 + gb] = np.arange(gb)
        D_of_pos[p0:p0 + gb] = D
    GBs = np.array([gb for _, gb, _ in plan], dtype=np.int64)
    Ds = np.array([D for _, _, D in plan], dtype=np.int64)
    off = np.concatenate([[0], np.cumsum(P * GBs * Ds)]).astype(np.int64)
    S = int(off[-1])  # slot rows per core

    pos = np.arange(n_pad)
    block = pos // P
    core_of = block % n_cores
    p_of = block // n_cores
    row = pos % P
    gg = g_of_pos[p_of]
    Dg = Ds[gg]
    # s-major scatter layout within a group; re-packed to interleaved below
    base = off[gg] + row * (GBs[gg] * Dg) + sub_of_pos[p_of] * Dg

    edge_order = np.argsort(dst, kind="stable")
    starts = np.concatenate([[0], np.cumsum(deg)])

    pp = np.repeat(pos, degs_o)
    cum0 = np.concatenate([[0], np.cumsum(degs_o)])[:-1]
    d_idx = np.arange(n_edges) - np.repeat(cum0, degs_o)
    node_of_pp = order[pp]
    edge_ids = edge_order[starts[node_of_pp] + d_idx]
    slot_global = core_of[pp] * S + base[pp] + d_idx

    kp = np.zeros((n_cores * S, 128), dtype=np.float32)
    kp[slot_global] = key[:, PERM_K][edge_ids]
    vp = np.zeros((n_cores * S, 96), dtype=np.float32)
    vp[slot_global] = value.reshape(n_edges, 96)[:, PERM_V][edge_ids]
    kp = kp.reshape(n_cores, S, 128)
    vp = vp.reshape(n_cores, S, 96)

    # Re-pack each group's per-node block from (s, d, feat) to the
    # innermost-interleaved device layouts:
    #   keys   (kk, d, h, s)   values (d, cx, h, s)
    for gi, (p0, gb, D) in enumerate(plan):
        s0, s1 = int(off[gi]), int(off[gi + 1])
        kb = kp[:, s0:s1].reshape(n_cores, P, gb, D, 16, 8)
        kp[:, s0:s1] = np.ascontiguousarray(
            kb.transpose(0, 1, 4, 3, 5, 2)).reshape(n_cores, s1 - s0, 128)
        vb = vp[:, s0:s1].reshape(n_cores, P, gb, D, 12, 8)
        vp[:, s0:s1] = np.ascontiguousarray(
            vb.transpose(0, 1, 3, 4, 5, 2)).reshape(n_cores, s1 - s0, 96)

    qfull = np.concatenate([query0, query1], axis=-1).reshape(n_nodes, 128)[:, PERM_K]
    q_pad = np.zeros((n_pad, 128), dtype=np.float32)
    q_pad[:n_nodes] = qfull
    q_sorted = q_pad[order].reshape(nb, P, 128)

    pc = (Dg - degs_o).astype(np.float32)
    zero_deg = degs_o == 0
    pc[zero_deg] = (Dg[zero_deg] - 1).astype(np.float32)
    pc_sorted = pc.reshape(nb, P)

    dt = BF16_NP
    kp = kp.astype(dt)
    vp = vp.astype(dt)
    in_maps = []
    for c in range(n_cores):
        qc = q_sorted[c::n_cores]              # [ng, P, 128], (kk, h) cols
        pcc = pc_sorted[c::n_cores]            # [ng, P]
        # queries per group in (kk, h, s) order; pad counts in (h, s) order
        q_c = np.zeros((P, ng * 128), dtype=np.float32)
        pc_c = np.zeros((P, ng * H), dtype=np.float32)
        for gi, (p0, gb, D) in enumerate(plan):
            qg = qc[p0:p0 + gb].reshape(gb, P, 16, 8)        # (s, P, kk, h)
            q_c[:, p0 * 128:(p0 + gb) * 128] = (
                qg.transpose(1, 2, 3, 0).reshape(P, gb * 128))
            pg = pcc[p0:p0 + gb]                             # (s, P)
            pc_c[:, p0 * H:(p0 + gb) * H] = np.repeat(
                pg.T[:, None, :], H, axis=1).reshape(P, H * gb)
        in_maps.append({"kp": kp[c], "vp": vp[c],
                        "q": q_c.astype(dt), "pc": pc_c})

    meta = dict(plan=plan, off=off, S=S, NG=len(plan), ng=ng, NB=nb,
                order=order, n_nodes=n_nodes, n_pad=n_pad)
    return in_maps, meta


def unshard_output(out_cores, meta):
    """out_cores: list of [128, ng*96] in (c, h, s) group order -> [N, 32, 3]."""
    nb, ng = meta["NB"], meta["ng"]
    plan = meta["plan"]
    n_cores = len(out_cores)
    order, n_nodes, n_pad = meta["order"], meta["n_nodes"], meta["n_pad"]
    out_sorted = np.zeros((nb, P, 96), dtype=np.float32)
    perm_v_inv = np.argsort(PERM_V)
    for c in range(n_cores):
        oc = out_cores[c]                      # [P, ng*96]
        po = np.zeros((ng, P, 96), dtype=np.float32)
        for gi, (p0, gb, D) in enumerate(plan):
            blk = oc[:, p0 * 96:(p0 + gb) * 96].reshape(P, 12, 8, gb)
            po[p0:p0 + gb] = blk.transpose(3, 0, 1, 2).reshape(gb, P, 96)
        out_sorted[c::n_cores] = po
    out_sorted = out_sorted.reshape(n_pad, 96)[:, perm_v_inv]
    out_full = np.zeros((n_nodes, 96), dtype=np.float32)
    mask = order < n_nodes
    out_full[order[mask]] = out_sorted[mask]
    return out_full.reshape(n_nodes, 32, 3)


# ---------------------------------------------------------------- bass kernel

def build(plan, S, ng, n_cores=N_CORES):
    NG = len(plan)
    off = np.concatenate(
        [[0], np.cumsum([P * gb * D for _, gb, D in plan])]).astype(np.int64)

    nc = bacc.Bacc("TRN2", target_bir_lowering=False, debug=False,
                   num_devices=n_cores)
    DT = mybir.dt.bfloat16
    kp = nc.declare_dram_parameter("kp", [S, 128], DT, isOutput=False)
    vp = nc.declare_dram_parameter("vp", [S, 96], DT, isOutput=False)
    q = nc.declare_dram_parameter("q", [P, ng * 128], DT, isOutput=False)
    pc = nc.declare_dram_parameter("pc", [P, ng * H], F32, isOutput=False)
    out = nc.declare_dram_parameter("out", [P, ng * 96], F32, isOutput=True)

    mult = mybir.AluOpType.mult
    add = mybir.AluOpType.add
    AX = mybir.AxisListType.X

    with tile.TileContext(nc) as tc:
        with tc.tile_pool(name="res", bufs=1) as res, \
             tc.tile_pool(name="ld", bufs=2) as ld, \
             tc.tile_pool(name="small", bufs=2) as small:
            q_sb = res.tile([P, ng * 128], DT)
            pc_sb = res.tile([P, ng * H], F32)
            out_sb = res.tile([P, ng * 96], F32)
            ss_all = res.tile([P, ng * H], F32)
            rs_all = res.tile([P, ng * H], F32)

            for g in range(NG):
                p0, GB, D = plan[g]
                W = GB * D           # slots per partition
                HS = H * GB          # interleaved (h, s) width
                s0 = int(off[g])
                # alternate k/v loads between the SP and ACT HWDGE queues
                eng_a = nc.sync if g % 2 == 0 else nc.scalar
                eng_b = nc.scalar if g % 2 == 0 else nc.sync
                # per-group query / pad-count slices first (tiny, FIFO per
                # queue: they must not sit behind the big k/v transfers)
                nc.scalar.dma_start(q_sb[:, p0 * 128:(p0 + GB) * 128],
                                    q[:, p0 * 128:(p0 + GB) * 128])
                nc.sync.dma_start(pc_sb[:, p0 * H:(p0 + GB) * H],
                                  pc[:, p0 * H:(p0 + GB) * H])
                kt = ld.tile([P, W * 128], DT, tag="kt", bufs=3)
                eng_a.dma_start(
                    kt[:], kp[s0:s0 + P * W, :].rearrange("(n w) f -> n (w f)", n=P))
                vt = ld.tile([P, W * 96], DT, tag="vt")
                eng_b.dma_start(
                    vt[:], vp[s0:s0 + P * W, :].rearrange("(n w) f -> n (w f)", n=P))

                # k *= q in place (content (kk, d, h, s); q is (kk, h, s) and
                # broadcasts over the middle d dim; bf16 2x, 3 free AP dims)
                qb = (q_sb[:, p0 * 128:(p0 + GB) * 128]
                      .rearrange("n (kk hs) -> n kk hs", kk=16)
                      .unsqueeze(2).broadcast_to([P, 16, D, HS]))
                k3 = kt[:].rearrange("n (kk d hs) -> n kk d hs", kk=16, d=D)
                nc.vector.tensor_tensor(out=k3, in0=k3, in1=qb, op=mult)

                # dk-halving tree: every level one fully contiguous add
                src, sw = kt, 128
                for X, tag in ((64, "t64"), (32, "t32"), (16, "t16")):
                    dstt = small.tile([P, W * X], DT, tag=tag)
                    nc.vector.tensor_tensor(
                        out=dstt[:], in0=src[:, :W * X], in1=src[:, W * X:W * sw],
                        op=add)
                    src, sw = dstt, X
                lg = small.tile([P, W * H], DT, tag="lg", bufs=3)
                nc.vector.tensor_tensor(
                    out=lg[:], in0=src[:, :W * H], in1=src[:, W * H:], op=add)

                # expw = exp(scale * logits)  (ScalarE; contiguous)
                ew = small.tile([P, W * H], DT, tag="ew", bufs=3)
                nc.scalar.activation(out=ew[:], in_=lg[:],
                                     func=mybir.ActivationFunctionType.Exp,
                                     scale=SCALE)

                # v *= expw in place (content (d, cx, h, s); expw is (d, h, s)
                # and broadcasts over the middle cx dim; bf16 2x, 3 free dims)
                eb = (ew[:].rearrange("n (d hs) -> n d hs", d=D)
                      .unsqueeze(2).broadcast_to([P, D, 12, HS]))
                v3 = vt[:].rearrange("n (d c hs) -> n d c hs", d=D, c=12)
                nc.vector.tensor_tensor(out=v3, in0=v3, in1=eb, op=mult)

                # segment sum of expw over d -> (h, s) per group: one flat
                # in-place halve (wv has consumed ew) then a strided reduce
                rs = D
                if rs % 2 == 0:
                    rs //= 2
                    nc.vector.tensor_tensor(
                        out=ew[:, :rs * HS], in0=ew[:, :rs * HS],
                        in1=ew[:, rs * HS:], op=add)
                nc.vector.tensor_reduce(
                    out=ss_all[:, p0 * H:(p0 + GB) * H],
                    in_=ew[:, :rs * HS].rearrange("n (d hs) -> n hs d", d=rs),
                    axis=AX, op=add)

                # d-reduction: halve in place over the leading d axis (fully
                # contiguous) while even, then strided reduce the remainder
                og = out_sb[:, p0 * 96:(p0 + GB) * 96]
                r = D
                FB = 96 * GB  # elems per d-layer
                while r > 1 and r % 2 == 0:
                    r //= 2
                    if r == 1:
                        nc.vector.tensor_tensor(
                            out=og, in0=vt[:, :FB], in1=vt[:, FB:2 * FB], op=add)
                    else:
                        nc.vector.tensor_tensor(
                            out=vt[:, :r * FB], in0=vt[:, :r * FB],
                            in1=vt[:, r * FB:2 * r * FB], op=add)
                # odd remainder: accumulate with flat 2x adds (a strided
                # tensor_reduce runs at ~1.8 cyc/elem, 3.5x slower)
                for j in range(1, r - 1):
                    nc.vector.tensor_tensor(
                        out=vt[:, :FB], in0=vt[:, :FB],
                        in1=vt[:, j * FB:(j + 1) * FB], op=add)
                if r > 1:
                    nc.vector.tensor_tensor(
                        out=og, in0=vt[:, :FB],
                        in1=vt[:, (r - 1) * FB:r * FB], op=add)

                # normalization + store per group: stores spread over the run
                # so only the last group's tail is exposed
                ca, cb = p0 * H, (p0 + GB) * H
                nc.vector.tensor_sub(out=ss_all[:, ca:cb], in0=ss_all[:, ca:cb],
                                     in1=pc_sb[:, ca:cb])
                nc.vector.reciprocal(out=rs_all[:, ca:cb], in_=ss_all[:, ca:cb])
                nc.vector.tensor_tensor(
                    out=og.rearrange("n (c hs) -> n c hs", c=12),
                    in0=og.rearrange("n (c hs) -> n c hs", c=12),
                    in1=(rs_all[:, ca:cb]
                         .unsqueeze(1).broadcast_to([P, 12, GB * H])),
                    op=mult)
                eng_a.dma_start(out[:, p0 * 96:(p0 + GB) * 96], og)

    nc.compile()
    return nc


# ---------------------------------------------------------------- entry point

LAST_RESULT = None  # BassKernelResults of the most recent run (for test harness)


def kernel(value, key, query0, query1, edge_index):
    global LAST_RESULT
    import os
    in_maps, meta = prepare(value, key, query0, query1, edge_index)
    nc = build(meta["plan"], meta["S"], meta["ng"])
    res = run_bass_kernel_spmd(nc, in_maps, list(range(N_CORES)),
                               tmpdir=os.environ.get("BASS_SPMD_TMPDIR"))
    LAST_RESULT = res
    out_cores = [res.results[c]["out"] for c in range(N_CORES)]
    return unshard_output(out_cores, meta)
